# revision 42
# baseline (speedup 1.0000x reference)
"""Trainium2 Bass kernel for the additive-attention module.

Computes, for inputs
    encoder_out [B=128, L=196, F=2048], hidden [B, H=1024],
    W_enc [F, A=1024], b_enc [A], W_hid [H, A], b_hid [A], W_full [A], b_full [1]:

    attn1 = encoder_out @ W_enc + b_enc          # (B, L, A)
    attn2 = hidden @ W_hid + b_hid               # (B, 1, A)
    attn  = relu(attn1 + attn2)
    scores = attn @ W_full + b_full              # (B, L)
    alpha = softmax(scores, axis=1)              # (B, L)
    context = einsum('blf,bl->bf', encoder_out, alpha)
    returns (context, alpha)

Sharding: data-parallel over batch across 8 NeuronCores (16 batches/core),
weights replicated.  Per core the kernel processes batches in pairs
(2*196 = 392 rows) so the fp32r matmuls run with a moving free dim >= 256
(full PE rate).  encoder_out tiles are loaded in natural [L, F] layout,
transposed on the PE (f on partitions) for the attn1 contraction over F;
the natural tiles are reused for the context contraction over L.

Note b_full only shifts the softmax logits, so it cannot affect either
output (softmax is shift invariant); it is accepted and ignored.
"""

import os
import sys

import numpy as np

for _p in ("/opt/trn_rl_repo", "/root/.axon_site/_ro/trn_rl_repo"):
    if os.path.isdir(_p) and _p not in sys.path:
        sys.path.insert(0, _p)

import concourse.bacc as bacc
import concourse.tile as tile
from concourse import masks, mybir
from concourse.bass_utils import run_bass_kernel_spmd
from concourse.tile import add_dep_helper

F32 = mybir.dt.float32
F32R = mybir.dt.float32r
AF = mybir.ActivationFunctionType
AX = mybir.AxisListType

N_CORES = 8
B, L, F, A, H = 128, 196, 2048, 1024, 1024
BC = B // N_CORES  # batches per core
NPAIR = BC // 2
L0, L1 = 128, L - 128  # 128 + 68 row split of one batch
FC, AC, HC = F // 128, A // 128, H // 128  # 16, 8, 8 chunks

_CACHE = {}


def _build():
    nc = bacc.Bacc(
        "TRN2",
        target_bir_lowering=False,
        debug=False,
        enable_asserts=False,
        num_devices=N_CORES,
    )
    enc = nc.dram_tensor("enc", [BC, L, F], F32R, kind="ExternalInput").ap()
    hid = nc.dram_tensor("hid", [BC, H], F32R, kind="ExternalInput").ap()
    w_enc = nc.dram_tensor("w_enc", [F, A], F32R, kind="ExternalInput").ap()
    w_hid = nc.dram_tensor("w_hid", [H, A], F32R, kind="ExternalInput").ap()
    b_enc = nc.dram_tensor("b_enc", [A], F32, kind="ExternalInput").ap()
    b_hid = nc.dram_tensor("b_hid", [A], F32, kind="ExternalInput").ap()
    w_full = nc.dram_tensor("w_full", [A], F32R, kind="ExternalInput").ap()
    ctx_out = nc.dram_tensor("ctx_out", [BC, F], F32, kind="ExternalOutput").ap()
    alpha_out = nc.dram_tensor("alpha_out", [BC, L], F32, kind="ExternalOutput").ap()

    with tile.TileContext(nc) as tc:
        _emit(nc, tc, enc, hid, w_enc, w_hid, b_enc, b_hid, w_full, ctx_out, alpha_out)
    nc.compile()
    return nc


def _emit(nc, tc, enc, hid, w_enc, w_hid, b_enc, b_hid, w_full, ctx_out, alpha_out):
    R = 2 * L  # rows per pair
    with tc.tile_pool(name="const", bufs=1) as const:
        ident_f = const.tile([128, 128], F32)
        masks.make_identity(nc, ident_f[:])
        ident = const.tile([128, 128], F32R)
        nc.vector.tensor_copy(ident[:], ident_f[:])

        with (
            tc.tile_pool(name="encp", bufs=4) as encp,
            tc.tile_pool(name="encTp", bufs=16) as enctp,
            tc.tile_pool(name="attnp", bufs=3) as attnp,
            tc.tile_pool(name="smp", bufs=2) as smp,
            tc.tile_pool(name="setup", bufs=1) as sp,
            tc.tile_pool(name="ps_a", bufs=3, space="PSUM") as ps_a,
            tc.tile_pool(name="ps_tr", bufs=2, space="PSUM") as ps_tr,
            tc.tile_pool(name="ps_m", bufs=3, space="PSUM") as ps_m,
        ):

            def load_pair(pair):
                # e128 tiles on the sync HWDGE FIFO, e68 tiles on the GPSIMD
                # SWDGE FIFO — two independent queues halve delivery latency
                b0 = 2 * pair
                e128, e68 = [], []
                for j in range(2):
                    t1 = encp.tile([128, F], F32R, tag="e128")
                    nc.sync.dma_start(t1[:], enc[b0 + j, 0:L0, :])
                    e128.append(t1)
                for j in range(2):
                    t2 = encp.tile([L1, F], F32R, tag="e68")
                    nc.gpsimd.dma_start(t2[:], enc[b0 + j, L0:L, :])
                    e68.append(t2)
                return e128, e68

            # ---- startup: tiny hid first, then pair-0 enc, then weights.
            # W_enc rides the GPSIMD (SWDGE) FIFO so it streams in parallel
            # with the sync FIFO's enc/W_hid traffic.
            hid_nat = sp.tile([BC, H], F32R)
            nc.sync.dma_start(hid_nat[:], hid)
            pair_enc = {0: load_pair(0)}

            # W_enc chunks pace pair-0's attn1: stripe them across both DMA
            # FIFOs (even->sync, odd->SWDGE); W_hid follows on the sync FIFO
            w_sb = []
            for fc in range(FC):
                w = const.tile([128, A], F32R, tag=f"we{fc}")
                q = nc.sync if fc % 2 == 0 else nc.gpsimd
                q.dma_start(w[:], w_enc[fc * 128 : (fc + 1) * 128, :])
                w_sb.append(w)
            whid_sb = []
            for hc in range(HC):
                w = sp.tile([128, A], F32R, tag=f"wh{hc}")
                nc.sync.dma_start(w[:], w_hid[hc * 128 : (hc + 1) * 128, :])
                whid_sb.append(w)
            wfull_sb = const.tile([128, AC], F32R)
            nc.sync.dma_start(wfull_sb[:], w_full.rearrange("(c p) -> p c", p=128))
            bias1 = const.tile([128, AC], F32)
            nc.sync.dma_start(bias1[:], b_enc.rearrange("(c p) -> p c", p=128))
            bias2 = const.tile([128, AC], F32)
            nc.sync.dma_start(bias2[:], b_hid.rearrange("(c p) -> p c", p=128))
            bias_eh = const.tile([128, AC], F32)
            nc.vector.tensor_add(bias_eh[:], bias1[:], bias2[:])

            attn2t = const.tile([128, AC, BC], F32)
            hidt = sp.tile([128, HC, BC], F32R)

            def emit_hidt_attn2():
                # attn2T[p, ac, b] = (hidden @ W_hid)[b, ac*128+p] + biases
                for hc in range(HC):
                    pt = ps_tr.tile([128, BC], F32R, tag="tr")
                    nc.tensor.transpose(
                        pt[:], hid_nat[:, hc * 128 : (hc + 1) * 128], ident[0:BC, 0:BC]
                    )
                    nc.vector.tensor_copy(hidt[:, hc, :], pt[:])
                for ac in range(AC):
                    pa = ps_m.tile([128, BC], F32, tag="m")
                    for hc in range(HC):
                        nc.tensor.matmul(
                            pa[:],
                            whid_sb[hc][:, ac * 128 : (ac + 1) * 128],
                            hidt[:, hc, :],
                            start=(hc == 0),
                            stop=(hc == HC - 1),
                        )
                    nc.scalar.activation(
                        attn2t[:, ac, :], pa[:], AF.Identity,
                        bias=bias_eh[:, ac : ac + 1],
                    )

            def emit_transposes(pair):
                """encT[fc][p, r] = enc[b(r), l(r), fc*128+p], r over 392 rows"""
                e128, e68 = pair_enc[pair]
                enct = []
                last = None
                for fc in range(FC):
                    fs = slice(fc * 128, (fc + 1) * 128)
                    ptr = ps_tr.tile([128, R], F32R, tag="tr")
                    nc.tensor.transpose(ptr[:, 0:L0], e128[0][:, fs], ident[:])
                    nc.tensor.transpose(ptr[:, L0:L], e68[0][:, fs], ident[0:L1, 0:L1])
                    nc.tensor.transpose(ptr[:, L : L + L0], e128[1][:, fs], ident[:])
                    last = nc.tensor.transpose(
                        ptr[:, L + L0 : R], e68[1][:, fs], ident[0:L1, 0:L1]
                    )
                    et = enctp.tile([128, R], F32R)
                    if fc % 2 == 0:
                        nc.scalar.copy(et[:], ptr[:])
                    else:
                        nc.vector.tensor_copy(et[:], ptr[:])
                    enct.append(et)
                return enct, last

            def emit_attn_scores_softmax(pair, enct):
                """attn1 -> relu -> scores -> softmax, all tall work on PE,
                softmax entirely on partition 0 (no partition shifts)."""
                b0 = 2 * pair
                psc = ps_m.tile([1, R], F32, tag="m")

                def emit_scores(ac, at):
                    nc.tensor.matmul(
                        psc[:],
                        wfull_sb[:, ac : ac + 1],
                        at[:],
                        start=(ac == 0),
                        stop=(ac == AC - 1),
                    )

                prev = None
                for ac in range(AC):
                    pa = ps_a.tile([128, R], F32)
                    for fc in range(FC):
                        nc.tensor.matmul(
                            pa[:],
                            w_sb[fc][:, ac * 128 : (ac + 1) * 128],
                            enct[fc][:],
                            start=(fc == 0),
                            stop=(fc == FC - 1),
                        )
                    at = attnp.tile([128, R], F32R)
                    # relu halves split across ACT and DVE so both batches'
                    # evacuations run in parallel
                    nc.scalar.activation(
                        at[:, 0:L],
                        pa[:, 0:L],
                        AF.Relu,
                        bias=attn2t[:, ac, b0 : b0 + 1],
                    )
                    nc.vector.tensor_scalar(
                        at[:, L:R],
                        pa[:, L:R],
                        attn2t[:, ac, b0 + 1 : b0 + 2],
                        0.0,
                        op0=mybir.AluOpType.add,
                        op1=mybir.AluOpType.max,
                    )
                    # scores lag one ac so attn1(ac+1) hides the relu latency
                    if prev is not None:
                        emit_scores(ac - 1, prev)
                    prev = at
                emit_scores(AC - 1, prev)

                # softmax on the [1, 392] scores row, reading PSUM directly.
                # No max subtraction: scores are O(1) for this problem's
                # scale (exp is safe), and softmax is shift-invariant.
                exps = smp.tile([1, R], F32, tag="exps")
                sume = smp.tile([1, 2], F32, tag="sume")
                for j in range(2):
                    js = slice(j * L, (j + 1) * L)
                    nc.scalar.activation(
                        exps[:, js],
                        psc[:, js],
                        AF.Exp,
                        accum_out=sume[:, j : j + 1],
                    )
                rec = smp.tile([1, 2], F32, tag="rec")
                nc.vector.reciprocal(rec[:], sume[:])
                # normalized alpha only feeds the alpha output; the context
                # path folds 1/sum into the alphaT matmul instead
                alr = smp.tile([1, R], F32, tag="al")
                for j in range(2):
                    js = slice(j * L, (j + 1) * L)
                    nc.vector.tensor_scalar_mul(alr[:, js], exps[:, js], rec[:, j : j + 1])
                    # issue on the ACT sequencer's HWDGE FIFO so the sync
                    # FIFO's enc prefetches are never queued behind this
                    nc.scalar.dma_start(alpha_out[b0 + j : b0 + j + 1, :], alr[:, js])
                return b0, exps, rec

            def emit_ctx(state, order_after=None):
                """context[b] = alpha[b] @ enc[b] (contraction over l on PE).
                Ordered after the NEXT pair's transposes so the PE reaches it
                well after this pair's softmax chain has finished."""
                b0, exps, rec = state
                pair = b0 // 2
                e128, e68 = pair_enc.pop(pair)
                for j in range(2):
                    # alphaT column vectors via tiny plain-fp32 matmuls that
                    # also fold in the softmax normalization:
                    # alphaT[l, 0] = exps[0, l] * (1/sum)
                    tp0 = ps_m.tile([128, 1], F32, tag="m")
                    mm0 = nc.tensor.matmul(
                        tp0[:],
                        exps[:, j * L : j * L + L0],
                        rec[:, j : j + 1],
                        start=True,
                        stop=True,
                    )
                    if order_after is not None:
                        # ordering-only edge: keep this behind the next
                        # pair's transposes in the PE's in-order stream
                        add_dep_helper(
                            mm0.ins, order_after.ins, sync=False,
                            reason="ctx after next-pair transposes",
                        )
                        order_after = None
                    tp1 = ps_m.tile([L1, 1], F32, tag="m")
                    nc.tensor.matmul(
                        tp1[:],
                        exps[:, j * L + L0 : (j + 1) * L],
                        rec[:, j : j + 1],
                        start=True,
                        stop=True,
                    )
                    alt0 = smp.tile([128, 1], F32R, tag="alt0")
                    nc.vector.tensor_copy(alt0[:], tp0[:])
                    alt1 = smp.tile([L1, 1], F32R, tag="alt1")
                    nc.vector.tensor_copy(alt1[:], tp1[:])

                    for ft in range(4):
                        fs = slice(ft * 512, (ft + 1) * 512)
                        pc = ps_m.tile([1, 512], F32, tag="m")
                        nc.tensor.matmul(
                            pc[:], alt0[:], e128[j][:, fs], start=True, stop=False
                        )
                        nc.tensor.matmul(
                            pc[:], alt1[:], e68[j][:, fs], start=False, stop=True
                        )
                        cbs = smp.tile([1, 512], F32, tag="cbs")
                        nc.scalar.copy(cbs[:], pc[:])
                        nc.scalar.dma_start(
                            ctx_out[b0 + j : b0 + j + 1, fs], cbs[:]
                        )

            # Pipeline: ... attn1/scores(p), [load(p+1), transposes(p+1)],
            # ctx(p), attn1/scores(p+1) ... — pair p+1's transposes separate
            # scores(p) from ctx(p) in the PE's in-order stream, hiding the
            # softmax-chain latency; enc slots for load(p+1) are recycled
            # from pair p-1, whose ctx preceded load(p+1) in emission.
            enct, _ = emit_transposes(0)
            emit_hidt_attn2()
            for pair in range(NPAIR):
                st = emit_attn_scores_softmax(pair, enct)
                last_tr = None
                if pair + 1 < NPAIR:
                    pair_enc[pair + 1] = load_pair(pair + 1)
                    enct, last_tr = emit_transposes(pair + 1)
                emit_ctx(st, order_after=last_tr)


def _get_nc():
    if "nc" not in _CACHE:
        _CACHE["nc"] = _build()
    return _CACHE["nc"]


def kernel(encoder_out, hidden, W_enc, b_enc, W_hid, b_hid, W_full, b_full):
    nc = _get_nc()
    encoder_out = np.ascontiguousarray(encoder_out, dtype=np.float32)
    hidden = np.ascontiguousarray(hidden, dtype=np.float32)
    shared = {
        "w_enc": np.ascontiguousarray(W_enc, dtype=np.float32),
        "w_hid": np.ascontiguousarray(W_hid, dtype=np.float32),
        "b_enc": np.ascontiguousarray(b_enc, dtype=np.float32),
        "b_hid": np.ascontiguousarray(b_hid, dtype=np.float32),
        "w_full": np.ascontiguousarray(W_full, dtype=np.float32),
    }
    in_maps = []
    for c in range(N_CORES):
        sl = slice(c * BC, (c + 1) * BC)
        in_maps.append(
            {
                "enc": np.ascontiguousarray(encoder_out[sl]),
                "hid": np.ascontiguousarray(hidden[sl]),
                **shared,
            }
        )
    try:
        res = run_bass_kernel_spmd(nc, in_maps, core_ids=list(range(N_CORES)))
    except Exception:
        # rare transient NRT_EXEC_UNIT_UNRECOVERABLE; one retry
        res = run_bass_kernel_spmd(nc, in_maps, core_ids=list(range(N_CORES)))
    ctx = np.concatenate([res.results[c]["ctx_out"] for c in range(N_CORES)], axis=0)
    alpha = np.concatenate(
        [res.results[c]["alpha_out"] for c in range(N_CORES)], axis=0
    )
    return ctx, alpha


# revision 49
# speedup vs baseline: 1.0819x; 1.0819x over previous
"""Trainium2 Bass kernel for the additive-attention module.

Computes, for inputs
    encoder_out [B=128, L=196, F=2048], hidden [B, H=1024],
    W_enc [F, A=1024], b_enc [A], W_hid [H, A], b_hid [A], W_full [A], b_full [1]:

    attn1 = encoder_out @ W_enc + b_enc          # (B, L, A)
    attn2 = hidden @ W_hid + b_hid               # (B, 1, A)
    attn  = relu(attn1 + attn2)
    scores = attn @ W_full + b_full              # (B, L)
    alpha = softmax(scores, axis=1)              # (B, L)
    context = einsum('blf,bl->bf', encoder_out, alpha)
    returns (context, alpha)

Sharding: data-parallel over batch across 8 NeuronCores (16 batches/core),
weights replicated.  Per core the kernel processes batches in pairs
(2*196 = 392 rows) so the fp32r matmuls run with a moving free dim >= 256
(full PE rate).  encoder_out tiles are loaded in natural [L, F] layout,
transposed on the PE (f on partitions) for the attn1 contraction over F;
the natural tiles are reused for the context contraction over L.

Note b_full only shifts the softmax logits, so it cannot affect either
output (softmax is shift invariant); it is accepted and ignored.
"""

import os
import sys

import numpy as np

for _p in ("/opt/trn_rl_repo", "/root/.axon_site/_ro/trn_rl_repo"):
    if os.path.isdir(_p) and _p not in sys.path:
        sys.path.insert(0, _p)

import concourse.bacc as bacc
import concourse.tile as tile
from concourse import masks, mybir
from concourse.bass_utils import run_bass_kernel_spmd
from concourse.tile import add_dep_helper

F32 = mybir.dt.float32
F32R = mybir.dt.float32r
AF = mybir.ActivationFunctionType
AX = mybir.AxisListType

N_CORES = 8
B, L, F, A, H = 128, 196, 2048, 1024, 1024
BC = B // N_CORES  # batches per core
NPAIR = BC // 2
L0, L1 = 128, L - 128  # 128 + 68 row split of one batch
FC, AC, HC = F // 128, A // 128, H // 128  # 16, 8, 8 chunks

_CACHE = {}


def _build():
    nc = bacc.Bacc(
        "TRN2",
        target_bir_lowering=False,
        debug=False,
        enable_asserts=False,
        num_devices=N_CORES,
    )
    enc = nc.dram_tensor("enc", [BC, L, F], F32R, kind="ExternalInput").ap()
    hid = nc.dram_tensor("hid", [BC, H], F32R, kind="ExternalInput").ap()
    w_enc = nc.dram_tensor("w_enc", [F, A], F32R, kind="ExternalInput").ap()
    w_hid = nc.dram_tensor("w_hid", [H, A], F32R, kind="ExternalInput").ap()
    b_enc = nc.dram_tensor("b_enc", [A], F32, kind="ExternalInput").ap()
    b_hid = nc.dram_tensor("b_hid", [A], F32, kind="ExternalInput").ap()
    w_full = nc.dram_tensor("w_full", [A], F32R, kind="ExternalInput").ap()
    ctx_out = nc.dram_tensor("ctx_out", [BC, F], F32, kind="ExternalOutput").ap()
    alpha_out = nc.dram_tensor("alpha_out", [BC, L], F32, kind="ExternalOutput").ap()

    with tile.TileContext(nc) as tc:
        _emit(nc, tc, enc, hid, w_enc, w_hid, b_enc, b_hid, w_full, ctx_out, alpha_out)
    nc.compile()
    return nc


def _emit(nc, tc, enc, hid, w_enc, w_hid, b_enc, b_hid, w_full, ctx_out, alpha_out):
    R = 2 * L  # rows per pair
    with tc.tile_pool(name="const", bufs=1) as const:
        ident_f = const.tile([128, 128], F32)
        masks.make_identity(nc, ident_f[:])
        ident = const.tile([128, 128], F32R)
        nc.vector.tensor_copy(ident[:], ident_f[:])

        with (
            tc.tile_pool(name="encp", bufs=4) as encp,
            tc.tile_pool(name="encTp", bufs=16) as enctp,
            tc.tile_pool(name="attnp", bufs=4) as attnp,
            tc.tile_pool(name="smp", bufs=2) as smp,
            tc.tile_pool(name="setup", bufs=1) as sp,
            tc.tile_pool(name="ps_a", bufs=3, space="PSUM") as ps_a,
            tc.tile_pool(name="ps_tr", bufs=2, space="PSUM") as ps_tr,
            tc.tile_pool(name="ps_m", bufs=3, space="PSUM") as ps_m,
        ):

            def load_pair(pair):
                # e128 tiles on the sync HWDGE FIFO, e68 tiles on the GPSIMD
                # SWDGE FIFO — two independent queues halve delivery latency
                b0 = 2 * pair
                e128, e68 = [], []
                for j in range(2):
                    t1 = encp.tile([128, F], F32R, tag="e128")
                    nc.sync.dma_start(t1[:], enc[b0 + j, 0:L0, :])
                    e128.append(t1)
                for j in range(2):
                    t2 = encp.tile([L1, F], F32R, tag="e68")
                    nc.gpsimd.dma_start(t2[:], enc[b0 + j, L0:L, :])
                    e68.append(t2)
                return e128, e68

            # ---- startup: tiny hid first, then pair-0 enc, then weights.
            # W_enc rides the GPSIMD (SWDGE) FIFO so it streams in parallel
            # with the sync FIFO's enc/W_hid traffic.
            hid_nat = sp.tile([BC, H], F32R)
            nc.sync.dma_start(hid_nat[:], hid)
            pair_enc = {0: load_pair(0)}

            # W_enc chunks pace pair-0's attn1: stripe them across both DMA
            # FIFOs (even->sync, odd->SWDGE); W_hid follows on the sync FIFO
            w_sb = []
            for fc in range(FC):
                w = const.tile([128, A], F32R, tag=f"we{fc}")
                q = nc.sync if fc % 2 == 0 else nc.gpsimd
                q.dma_start(w[:], w_enc[fc * 128 : (fc + 1) * 128, :])
                w_sb.append(w)
            whid_sb = []
            for hc in range(HC):
                w = sp.tile([128, A], F32R, tag=f"wh{hc}")
                nc.sync.dma_start(w[:], w_hid[hc * 128 : (hc + 1) * 128, :])
                whid_sb.append(w)
            wfull_sb = const.tile([128, AC], F32R)
            nc.sync.dma_start(wfull_sb[:], w_full.rearrange("(c p) -> p c", p=128))
            bias1 = const.tile([128, AC], F32)
            nc.sync.dma_start(bias1[:], b_enc.rearrange("(c p) -> p c", p=128))
            bias2 = const.tile([128, AC], F32)
            nc.sync.dma_start(bias2[:], b_hid.rearrange("(c p) -> p c", p=128))
            bias_eh = const.tile([128, AC], F32)
            nc.vector.tensor_add(bias_eh[:], bias1[:], bias2[:])

            attn2t = const.tile([128, AC, BC], F32)
            hidt = sp.tile([128, HC, BC], F32R)

            def emit_hidt_attn2():
                # attn2T[p, ac, b] = (hidden @ W_hid)[b, ac*128+p] + biases
                for hc in range(HC):
                    pt = ps_tr.tile([128, BC], F32R, tag="tr")
                    nc.tensor.transpose(
                        pt[:], hid_nat[:, hc * 128 : (hc + 1) * 128], ident[0:BC, 0:BC]
                    )
                    nc.vector.tensor_copy(hidt[:, hc, :], pt[:])
                for ac in range(AC):
                    pa = ps_m.tile([128, BC], F32, tag="m")
                    for hc in range(HC):
                        nc.tensor.matmul(
                            pa[:],
                            whid_sb[hc][:, ac * 128 : (ac + 1) * 128],
                            hidt[:, hc, :],
                            start=(hc == 0),
                            stop=(hc == HC - 1),
                        )
                    nc.scalar.activation(
                        attn2t[:, ac, :], pa[:], AF.Identity,
                        bias=bias_eh[:, ac : ac + 1],
                    )

            def emit_transposes(pair):
                """encT[fc][p, r] = enc[b(r), l(r), fc*128+p], r over 392 rows"""
                e128, e68 = pair_enc[pair]
                enct = []
                last = None
                for fc in range(FC):
                    fs = slice(fc * 128, (fc + 1) * 128)
                    ptr = ps_tr.tile([128, R], F32R, tag="tr")
                    nc.tensor.transpose(ptr[:, 0:L0], e128[0][:, fs], ident[:])
                    nc.tensor.transpose(ptr[:, L0:L], e68[0][:, fs], ident[0:L1, 0:L1])
                    nc.tensor.transpose(ptr[:, L : L + L0], e128[1][:, fs], ident[:])
                    last = nc.tensor.transpose(
                        ptr[:, L + L0 : R], e68[1][:, fs], ident[0:L1, 0:L1]
                    )
                    et = enctp.tile([128, R], F32R)
                    if fc % 2 == 0:
                        nc.scalar.copy(et[:], ptr[:])
                    else:
                        nc.vector.tensor_copy(et[:], ptr[:])
                    enct.append(et)
                return enct, last

            def emit_attn_scores_softmax(pair, enct):
                """attn1 -> relu -> scores -> softmax, all tall work on PE,
                softmax entirely on partition 0 (no partition shifts)."""
                b0 = 2 * pair
                psc = ps_m.tile([1, R], F32, tag="m")

                def emit_scores(ac, at):
                    nc.tensor.matmul(
                        psc[:],
                        wfull_sb[:, ac : ac + 1],
                        at[:],
                        start=(ac == 0),
                        stop=(ac == AC - 1),
                    )

                def attn1_accumulations():
                    """Yield (ac, psum) in completion order.  Pair 0 runs
                    fc-outer in groups of two a-chunks so the matmuls stream
                    with W_enc chunk DMA arrivals instead of stalling on the
                    full 8 MB load."""
                    if pair != 0:
                        for ac in range(AC):
                            pa = ps_a.tile([128, R], F32, tag="pa")
                            for fc in range(FC):
                                nc.tensor.matmul(
                                    pa[:],
                                    w_sb[fc][:, ac * 128 : (ac + 1) * 128],
                                    enct[fc][:],
                                    start=(fc == 0),
                                    stop=(fc == FC - 1),
                                )
                            yield ac, pa
                    else:
                        for g in range(AC // 2):
                            pa2 = [
                                ps_a.tile([128, R], F32, tag="pa", name=f"pa_{g}_{k}")
                                for k in range(2)
                            ]
                            for fc in range(FC):
                                for k in range(2):
                                    ac = 2 * g + k
                                    nc.tensor.matmul(
                                        pa2[k][:],
                                        w_sb[fc][:, ac * 128 : (ac + 1) * 128],
                                        enct[fc][:],
                                        start=(fc == 0),
                                        stop=(fc == FC - 1),
                                    )
                            yield 2 * g, pa2[0]
                            yield 2 * g + 1, pa2[1]

                pending = []
                for ac, pa in attn1_accumulations():
                    at = attnp.tile([128, R], F32R)
                    # relu halves split across ACT and DVE so both batches'
                    # evacuations run in parallel
                    nc.scalar.activation(
                        at[:, 0:L],
                        pa[:, 0:L],
                        AF.Relu,
                        bias=attn2t[:, ac, b0 : b0 + 1],
                    )
                    nc.vector.tensor_scalar(
                        at[:, L:R],
                        pa[:, L:R],
                        attn2t[:, ac, b0 + 1 : b0 + 2],
                        0.0,
                        op0=mybir.AluOpType.add,
                        op1=mybir.AluOpType.max,
                    )
                    # scores lag two a-chunks so attn1 work hides relu latency
                    pending.append((ac, at))
                    if len(pending) >= 3:
                        emit_scores(*pending.pop(0))
                for item in pending:
                    emit_scores(*item)

                # softmax on the [1, 392] scores row, reading PSUM directly.
                # No max subtraction: scores are O(1) for this problem's
                # scale (exp is safe), and softmax is shift-invariant.
                exps = smp.tile([1, R], F32, tag="exps")
                sume = smp.tile([1, 2], F32, tag="sume")
                for j in range(2):
                    js = slice(j * L, (j + 1) * L)
                    nc.scalar.activation(
                        exps[:, js],
                        psc[:, js],
                        AF.Exp,
                        accum_out=sume[:, j : j + 1],
                    )
                rec = smp.tile([1, 2], F32, tag="rec")
                nc.vector.reciprocal(rec[:], sume[:])
                # normalized alpha only feeds the alpha output; the context
                # path folds 1/sum into the alphaT matmul instead
                alr = smp.tile([1, R], F32, tag="al")
                for j in range(2):
                    js = slice(j * L, (j + 1) * L)
                    nc.vector.tensor_scalar_mul(alr[:, js], exps[:, js], rec[:, j : j + 1])
                    # issue on the ACT sequencer's HWDGE FIFO so the sync
                    # FIFO's enc prefetches are never queued behind this
                    nc.scalar.dma_start(alpha_out[b0 + j : b0 + j + 1, :], alr[:, js])
                return b0, exps, rec

            def emit_ctx(state, order_after=None):
                """context[b] = alpha[b] @ enc[b] (contraction over l on PE).
                Ordered after the NEXT pair's transposes so the PE reaches it
                well after this pair's softmax chain has finished."""
                b0, exps, rec = state
                pair = b0 // 2
                e128, e68 = pair_enc.pop(pair)
                for j in range(2):
                    # alphaT column vectors via tiny plain-fp32 matmuls that
                    # also fold in the softmax normalization:
                    # alphaT[l, 0] = exps[0, l] * (1/sum)
                    tp0 = ps_m.tile([128, 1], F32, tag="m")
                    mm0 = nc.tensor.matmul(
                        tp0[:],
                        exps[:, j * L : j * L + L0],
                        rec[:, j : j + 1],
                        start=True,
                        stop=True,
                    )
                    if order_after is not None:
                        # ordering-only edge: keep this behind the next
                        # pair's transposes in the PE's in-order stream
                        add_dep_helper(
                            mm0.ins, order_after.ins, sync=False,
                            reason="ctx after next-pair transposes",
                        )
                        order_after = None
                    tp1 = ps_m.tile([L1, 1], F32, tag="m")
                    nc.tensor.matmul(
                        tp1[:],
                        exps[:, j * L + L0 : (j + 1) * L],
                        rec[:, j : j + 1],
                        start=True,
                        stop=True,
                    )
                    alt0 = smp.tile([128, 1], F32R, tag="alt0")
                    nc.vector.tensor_copy(alt0[:], tp0[:])
                    alt1 = smp.tile([L1, 1], F32R, tag="alt1")
                    nc.vector.tensor_copy(alt1[:], tp1[:])

                    for ft in range(4):
                        fs = slice(ft * 512, (ft + 1) * 512)
                        pc = ps_m.tile([1, 512], F32, tag="m")
                        nc.tensor.matmul(
                            pc[:], alt0[:], e128[j][:, fs], start=True, stop=False
                        )
                        nc.tensor.matmul(
                            pc[:], alt1[:], e68[j][:, fs], start=False, stop=True
                        )
                        cbs = smp.tile([1, 512], F32, tag="cbs")
                        nc.scalar.copy(cbs[:], pc[:])
                        nc.scalar.dma_start(
                            ctx_out[b0 + j : b0 + j + 1, fs], cbs[:]
                        )

            # Pipeline: ... attn1/scores(p), [load(p+1), transposes(p+1)],
            # ctx(p), attn1/scores(p+1) ... — pair p+1's transposes separate
            # scores(p) from ctx(p) in the PE's in-order stream, hiding the
            # softmax-chain latency; enc slots for load(p+1) are recycled
            # from pair p-1, whose ctx preceded load(p+1) in emission.
            enct, _ = emit_transposes(0)
            emit_hidt_attn2()
            for pair in range(NPAIR):
                st = emit_attn_scores_softmax(pair, enct)
                last_tr = None
                if pair + 1 < NPAIR:
                    pair_enc[pair + 1] = load_pair(pair + 1)
                    enct, last_tr = emit_transposes(pair + 1)
                emit_ctx(st, order_after=last_tr)


def _get_nc():
    if "nc" not in _CACHE:
        _CACHE["nc"] = _build()
    return _CACHE["nc"]


def kernel(encoder_out, hidden, W_enc, b_enc, W_hid, b_hid, W_full, b_full):
    nc = _get_nc()
    encoder_out = np.ascontiguousarray(encoder_out, dtype=np.float32)
    hidden = np.ascontiguousarray(hidden, dtype=np.float32)
    shared = {
        "w_enc": np.ascontiguousarray(W_enc, dtype=np.float32),
        "w_hid": np.ascontiguousarray(W_hid, dtype=np.float32),
        "b_enc": np.ascontiguousarray(b_enc, dtype=np.float32),
        "b_hid": np.ascontiguousarray(b_hid, dtype=np.float32),
        "w_full": np.ascontiguousarray(W_full, dtype=np.float32),
    }
    in_maps = []
    for c in range(N_CORES):
        sl = slice(c * BC, (c + 1) * BC)
        in_maps.append(
            {
                "enc": np.ascontiguousarray(encoder_out[sl]),
                "hid": np.ascontiguousarray(hidden[sl]),
                **shared,
            }
        )
    try:
        res = run_bass_kernel_spmd(nc, in_maps, core_ids=list(range(N_CORES)))
    except Exception:
        # rare transient NRT_EXEC_UNIT_UNRECOVERABLE; one retry
        res = run_bass_kernel_spmd(nc, in_maps, core_ids=list(range(N_CORES)))
    ctx = np.concatenate([res.results[c]["ctx_out"] for c in range(N_CORES)], axis=0)
    alpha = np.concatenate(
        [res.results[c]["alpha_out"] for c in range(N_CORES)], axis=0
    )
    return ctx, alpha
